# revision 17
# baseline (speedup 1.0000x reference)
"""Trainium2 Bass kernel: ConsPosiEmb (positional-reset embedding lookup).

Semantics (matches the reference nn.Module):
  pos[b, j] = j - last_sep[b, j] + 2, where last_sep is the running max of
              indices of SEP tokens (token id 4), i.e. positions reset to 2
              at each SEP and count up;
  any token at/after the first PAD token (id 1) maps to table row 1, which
  is all zeros.
  out[b, j, :] = table[pos[b, j], :]        # table: [4098, 1024] f32

Device-side algorithm (one NeuronCore handles 4 of the 32 batch rows):
  1. Load tokens [4, 4096] to SBUF; compute in f32:
       sep_j   = (tok == 4) * j
       last    = running-max-scan(sep_j)          (tensor_tensor_scan)
       invbig  = running-max-scan((tok == 1) * 8192)
       gidx_f  = (j + 2) - last + invbig          # > 4097 at padded slots
  2. PE-transpose gidx_f [4, 4096] into column layout [128, 128]:
       ps[p, 4k + b] = gidx_f[b, 128k + p]
  3. Indirect-DMA gather (SWDGE, per-descriptor 4KB rows) from the table in
     HBM with bounds_check=4097, oob_is_err=False: padded slots are skipped
     (no HBM read traffic for the pad tail).
  4. Indirect-DMA scatter to the output with idx = b*4096 + 128k + p at
     valid slots and an out-of-bounds value at padded slots: the pad tail
     is never written and stays at the zero-initialized output contents
     (run_bass_kernel_spmd pre-zeroes ExternalOutput buffers).
This moves ~(valid fraction)*128MB instead of 128MB of HBM traffic/core.
"""

import os
import sys
from contextlib import ExitStack

import numpy as np

try:
    import concourse.bass as bass
except ImportError:  # fall back to the standard repo locations
    for _p in ("/opt/trn_rl_repo", "/root/.axon_site/_ro/trn_rl_repo"):
        if os.path.isdir(_p) and _p not in sys.path:
            sys.path.insert(0, _p)
    import concourse.bass as bass

import concourse.tile as tile
from concourse import bacc, bass_utils, mybir
from concourse.masks import make_identity

P = 128
PAD_IDX = 1
SEP_ID = 4
BIG = 8192.0  # added to gather idx at padded slots -> OOB -> read skipped
OUT_BIG = float(1 << 22)  # added to scatter idx at padded slots -> write skipped

# Full-problem dimensions (hardcoded per harness contract)
BSZ, SEQ, DIM = 32, 4096, 1024
NTAB = SEQ + 2  # 4098
NCORES = 8
RPC = BSZ // NCORES  # batch rows per core


def build_nc(rows=RPC, seq=SEQ, d=DIM, ntab=NTAB, kt=None, bufs=8,
             skip_pads=False, scatter_write=False):
    """Build the single-core SPMD Bass program.

    rows x seq int32 tokens -> [rows*seq, d] f32 embeddings.
    skip_pads: add BIG to gather idx at padded slots + bounds_check so the
        HW skips those reads (otherwise pads gather the zeroed table row 1).
    scatter_write: write via indirect scatter with OOB pad skip (requires
        skip_pads); otherwise plain DMA stores write every row.
    """
    assert not (scatter_write and not skip_pads)
    K = seq // P  # 128-token tiles per row
    assert seq % P == 0
    f32, i32 = mybir.dt.float32, mybir.dt.int32
    Alu = mybir.AluOpType

    nc = bacc.Bacc("TRN2", target_bir_lowering=False, debug=False)
    tok_d = nc.dram_tensor("tokens", [rows, seq], i32, kind="ExternalInput")
    tab_d = nc.dram_tensor("table", [ntab, d], f32, kind="ExternalInput")
    out_d = nc.dram_tensor("out", [rows * seq, d], f32, kind="ExternalOutput")

    with ExitStack() as ctx:
        tc = ctx.enter_context(tile.TileContext(nc))
        idxp = ctx.enter_context(tc.tile_pool(name="idx", bufs=1))
        psum_pool = ctx.enter_context(tc.tile_pool(name="ps", bufs=1, space="PSUM"))

        gidx_b, sidx_b = [], []
        # Scoped scratch: the [rows, seq] f32 temporaries are released
        # before the big data pool opens (SBUF address-space reuse).
        with tc.tile_pool(name="scratch", bufs=1) as scr:
            tok_i = scr.tile([rows, seq], i32)
            nc.sync.dma_start(tok_i[:], tok_d.ap())
            tokf = scr.tile([rows, seq], f32)
            nc.vector.tensor_copy(tokf[:], tok_i[:])

            jvec0 = scr.tile([rows, seq], f32)
            nc.gpsimd.iota(
                jvec0[:], [[1, seq]], base=0, channel_multiplier=0,
                allow_small_or_imprecise_dtypes=True,
            )
            # sep_j = (tok == SEP) * j
            sepj = scr.tile([rows, seq], f32)
            nc.vector.scalar_tensor_tensor(
                sepj[:], tokf[:], float(SEP_ID), jvec0[:],
                op0=Alu.is_equal, op1=Alu.mult,
            )
            # last_sep = running max of sep_j along the sequence
            lsep = scr.tile([rows, seq], f32)
            nc.vector.tensor_tensor_scan(
                lsep[:], sepj[:], sepj[:], 0.0, op0=Alu.max, op1=Alu.max
            )
            # invb = (tok == PAD) * BIG; invs = running max (sticky marker)
            invb = scr.tile([rows, seq], f32)
            nc.gpsimd.tensor_scalar(
                out=invb[:], in0=tokf[:], scalar1=float(PAD_IDX), scalar2=BIG,
                op0=Alu.is_equal, op1=Alu.mult,
            )
            # skip mode consumes invs as an f32 addend; plain mode as an
            # integer mask for copy_predicated (BIR requires int mask)
            invs = scr.tile([rows, seq], f32 if skip_pads else i32)
            nc.vector.tensor_tensor_scan(
                invs[:], invb[:], invb[:], 0.0, op0=Alu.max, op1=Alu.max
            )
            # gather idx (f32): ((j - last_sep) + 2), pads handled below
            gif0 = scr.tile([rows, seq], f32)
            nc.vector.tensor_tensor(gif0[:], jvec0[:], lsep[:], op=Alu.subtract)
            gif = scr.tile([rows, seq], f32)
            if skip_pads:
                # pads become > BIG -> skipped by bounds_check on the gather
                nc.vector.scalar_tensor_tensor(
                    gif[:], gif0[:], 2.0, invs[:], op0=Alu.add, op1=Alu.add
                )
            else:
                # pads become exactly 1 -> gather the zeroed table row
                nc.vector.tensor_scalar(
                    out=gif[:], in0=gif0[:], scalar1=2.0, scalar2=None,
                    op0=Alu.add,
                )
                one = idxp.tile([rows, 1], f32)
                nc.gpsimd.memset(one[:], 1.0)
                nc.vector.copy_predicated(
                    gif[:], invs[:], one[:].to_broadcast([rows, seq])
                )

            # transpose to column layout: ps[p, k*rows + b] = gif[b, k*P + p]
            ident = idxp.tile([rows, rows], f32)
            make_identity(nc, ident[:])
            ps = psum_pool.tile([P, K * rows], f32)
            for k in range(K):
                nc.tensor.transpose(
                    ps[:, k * rows:(k + 1) * rows],
                    gif[:, k * P:(k + 1) * P],
                    ident[:],
                )
            # De-interleave per batch row into contiguous [P, K] index tiles
            # (DMA offset APs must be contiguous in the last dim).
            ps3 = ps[:].rearrange("p (k b) -> p k b", b=rows)
            for b in range(rows):
                g = idxp.tile([P, K], i32, tag=f"gidx{b}")
                nc.vector.tensor_copy(g[:], ps3[:, :, b])
                gidx_b.append(g)
                if not scatter_write:
                    continue
                # scatter idx = (b*seq + k*P + p) + (gidx_f > BIG-1)*OUT_BIG
                sio = idxp.tile([P, K], i32, tag=f"sio{b}")
                nc.gpsimd.iota(
                    sio[:], [[P, K]], base=b * seq, channel_multiplier=1
                )
                mb = idxp.tile([P, K], i32, tag=f"mb{b}")
                nc.vector.tensor_scalar(
                    out=mb[:], in0=ps3[:, :, b], scalar1=BIG - 1.0,
                    scalar2=OUT_BIG, op0=Alu.is_gt, op1=Alu.mult,
                )
                s = idxp.tile([P, K], i32, tag=f"sidx{b}")
                nc.vector.tensor_tensor(s[:], sio[:], mb[:], op=Alu.add)
                sidx_b.append(s)

        data = ctx.enter_context(tc.tile_pool(name="data", bufs=bufs))

        # Per-token tiles: one [128, d] tile covers 128 consecutive output
        # rows (token j = b*seq + 128k + p on partition p). The indirect
        # offset AP is [128, 1]: the HW consumes exactly one index per
        # partition (one 4KB-row descriptor per partition).
        outv = out_d.ap().rearrange("(b k p) d -> b k p d", b=rows, p=P)
        for b in range(rows):
            for k in range(K):
                t = data.tile([P, d], f32)
                g_ap = gidx_b[b][:, k:k + 1]
                nc.gpsimd.indirect_dma_start(
                    out=t[:],
                    out_offset=None,
                    in_=tab_d.ap(),
                    in_offset=bass.IndirectOffsetOnAxis(ap=g_ap, axis=0),
                    bounds_check=ntab - 1 if skip_pads else None,
                    oob_is_err=not skip_pads,
                )
                if scatter_write:
                    s_ap = sidx_b[b][:, k:k + 1]
                    nc.gpsimd.indirect_dma_start(
                        out=out_d.ap(),
                        out_offset=bass.IndirectOffsetOnAxis(ap=s_ap, axis=0),
                        in_=t[:],
                        in_offset=None,
                        bounds_check=rows * seq - 1,
                        oob_is_err=False,
                    )
                else:
                    nc.sync.dma_start(outv[b, k], t[:])
    nc.compile()
    return nc


_nc_cache = {}

# Tuned configuration used by kernel()
KERNEL_CFG = dict(skip_pads=False, scatter_write=False)


def _get_nc(**cfg):
    key = tuple(sorted(cfg.items()))
    if key not in _nc_cache:
        _nc_cache[key] = build_nc(**cfg)
    return _nc_cache[key]


def run(input, weights, trace=False, **cfg):
    """Run the 8-core SPMD kernel; returns (output, BassKernelResults)."""
    tokens = np.ascontiguousarray(np.asarray(input).astype(np.int32))
    table = np.ascontiguousarray(np.asarray(weights, dtype=np.float32))
    assert tokens.shape == (BSZ, SEQ), tokens.shape
    assert table.shape == (NTAB, DIM), table.shape
    nc = _get_nc(**{**KERNEL_CFG, **cfg})
    in_maps = [
        {"tokens": np.ascontiguousarray(tokens[c * RPC:(c + 1) * RPC]),
         "table": table}
        for c in range(NCORES)
    ]
    res = bass_utils.run_bass_kernel_spmd(
        nc, in_maps, core_ids=list(range(NCORES)), trace=trace
    )
    out = np.concatenate(
        [r["out"].reshape(RPC, SEQ, DIM) for r in res.results], axis=0
    )
    return out, res


def kernel(input, weights):
    out, _ = run(input, weights)
    return out


# revision 18
# speedup vs baseline: 1.0897x; 1.0897x over previous
"""Trainium2 Bass kernel: ConsPosiEmb (positional-reset embedding lookup).

Semantics (matches the reference nn.Module):
  pos[b, j] = j - last_sep[b, j] + 2, where last_sep is the running max of
              indices of SEP tokens (token id 4), i.e. positions reset to 2
              at each SEP and count up;
  any token at/after the first PAD token (id 1) maps to table row 1, which
  is all zeros.
  out[b, j, :] = table[pos[b, j], :]        # table: [4098, 1024] f32

Device-side algorithm (one NeuronCore handles 4 of the 32 batch rows):
  1. Load tokens [4, 4096] to SBUF; compute in f32:
       sep_j   = (tok == 4) * j
       last    = running-max-scan(sep_j)          (tensor_tensor_scan)
       invbig  = running-max-scan((tok == 1) * 8192)
       gidx_f  = (j + 2) - last + invbig          # > 4097 at padded slots
  2. PE-transpose gidx_f [4, 4096] into column layout [128, 128]:
       ps[p, 4k + b] = gidx_f[b, 128k + p]
  3. Indirect-DMA gather (SWDGE, per-descriptor 4KB rows) from the table in
     HBM with bounds_check=4097, oob_is_err=False: padded slots are skipped
     (no HBM read traffic for the pad tail).
  4. Indirect-DMA scatter to the output with idx = b*4096 + 128k + p at
     valid slots and an out-of-bounds value at padded slots: the pad tail
     is never written and stays at the zero-initialized output contents
     (run_bass_kernel_spmd pre-zeroes ExternalOutput buffers).
This moves ~(valid fraction)*128MB instead of 128MB of HBM traffic/core.
"""

import os
import sys
from contextlib import ExitStack

import numpy as np

try:
    import concourse.bass as bass
except ImportError:  # fall back to the standard repo locations
    for _p in ("/opt/trn_rl_repo", "/root/.axon_site/_ro/trn_rl_repo"):
        if os.path.isdir(_p) and _p not in sys.path:
            sys.path.insert(0, _p)
    import concourse.bass as bass

import concourse.tile as tile
from concourse import bacc, bass_utils, mybir
from concourse.masks import make_identity

P = 128
PAD_IDX = 1
SEP_ID = 4
BIG = 8192.0  # added to gather idx at padded slots -> OOB -> read skipped
OUT_BIG = float(1 << 22)  # added to scatter idx at padded slots -> write skipped

# Full-problem dimensions (hardcoded per harness contract)
BSZ, SEQ, DIM = 32, 4096, 1024
NTAB = SEQ + 2  # 4098
NCORES = 8
RPC = BSZ // NCORES  # batch rows per core


def build_nc(rows=RPC, seq=SEQ, d=DIM, ntab=NTAB, kt=None, bufs=8,
             skip_pads=False, scatter_write=False):
    """Build the single-core SPMD Bass program.

    rows x seq int32 tokens -> [rows*seq, d] f32 embeddings.
    skip_pads: add BIG to gather idx at padded slots + bounds_check so the
        HW skips those reads (otherwise pads gather the zeroed table row 1).
    scatter_write: write via indirect scatter with OOB pad skip (requires
        skip_pads); otherwise plain DMA stores write every row.
    """
    assert not (scatter_write and not skip_pads)
    K = seq // P  # 128-token tiles per row
    assert seq % P == 0
    f32, i32 = mybir.dt.float32, mybir.dt.int32
    Alu = mybir.AluOpType

    nc = bacc.Bacc("TRN2", target_bir_lowering=False, debug=False)
    tok_d = nc.dram_tensor("tokens", [rows, seq], i32, kind="ExternalInput")
    tab_d = nc.dram_tensor("table", [ntab, d], f32, kind="ExternalInput")
    out_d = nc.dram_tensor("out", [rows * seq, d], f32, kind="ExternalOutput")

    with ExitStack() as ctx:
        tc = ctx.enter_context(tile.TileContext(nc))
        idxp = ctx.enter_context(tc.tile_pool(name="idx", bufs=1))
        psum_pool = ctx.enter_context(tc.tile_pool(name="ps", bufs=1, space="PSUM"))

        gidx_b, sidx_b = [], []
        # Scoped scratch: the [rows, seq] f32 temporaries are released
        # before the big data pool opens (SBUF address-space reuse).
        with tc.tile_pool(name="scratch", bufs=1) as scr:
            tok_i = scr.tile([rows, seq], i32)
            nc.sync.dma_start(tok_i[:], tok_d.ap())
            tokf = scr.tile([rows, seq], f32)
            nc.vector.tensor_copy(tokf[:], tok_i[:])

            jvec0 = scr.tile([rows, seq], f32)
            nc.gpsimd.iota(
                jvec0[:], [[1, seq]], base=0, channel_multiplier=0,
                allow_small_or_imprecise_dtypes=True,
            )
            # sep_j = (tok == SEP) * j
            sepj = scr.tile([rows, seq], f32)
            nc.vector.scalar_tensor_tensor(
                sepj[:], tokf[:], float(SEP_ID), jvec0[:],
                op0=Alu.is_equal, op1=Alu.mult,
            )
            # last_sep = running max of sep_j along the sequence
            lsep = scr.tile([rows, seq], f32)
            nc.vector.tensor_tensor_scan(
                lsep[:], sepj[:], sepj[:], 0.0, op0=Alu.max, op1=Alu.max
            )
            # invb = (tok == PAD) * BIG; invs = running max (sticky marker)
            invb = scr.tile([rows, seq], f32)
            nc.gpsimd.tensor_scalar(
                out=invb[:], in0=tokf[:], scalar1=float(PAD_IDX), scalar2=BIG,
                op0=Alu.is_equal, op1=Alu.mult,
            )
            # skip mode consumes invs as an f32 addend; plain mode as an
            # integer mask for copy_predicated (BIR requires int mask)
            invs = scr.tile([rows, seq], f32 if skip_pads else i32)
            nc.vector.tensor_tensor_scan(
                invs[:], invb[:], invb[:], 0.0, op0=Alu.max, op1=Alu.max
            )
            # gather idx (f32): ((j - last_sep) + 2), pads handled below
            gif0 = scr.tile([rows, seq], f32)
            nc.vector.tensor_tensor(gif0[:], jvec0[:], lsep[:], op=Alu.subtract)
            gif = scr.tile([rows, seq], f32)
            if skip_pads:
                # pads become > BIG -> skipped by bounds_check on the gather
                nc.vector.scalar_tensor_tensor(
                    gif[:], gif0[:], 2.0, invs[:], op0=Alu.add, op1=Alu.add
                )
            else:
                # pads become exactly 1 -> gather the zeroed table row
                nc.vector.tensor_scalar(
                    out=gif[:], in0=gif0[:], scalar1=2.0, scalar2=None,
                    op0=Alu.add,
                )
                one = idxp.tile([rows, 1], f32)
                nc.gpsimd.memset(one[:], 1.0)
                nc.vector.copy_predicated(
                    gif[:], invs[:], one[:].to_broadcast([rows, seq])
                )

            # transpose to column layout: ps[p, k*rows + b] = gif[b, k*P + p]
            ident = idxp.tile([rows, rows], f32)
            make_identity(nc, ident[:])
            ps = psum_pool.tile([P, K * rows], f32)
            for k in range(K):
                nc.tensor.transpose(
                    ps[:, k * rows:(k + 1) * rows],
                    gif[:, k * P:(k + 1) * P],
                    ident[:],
                )
            # De-interleave per batch row into contiguous [P, K] index tiles
            # (DMA offset APs must be contiguous in the last dim).
            ps3 = ps[:].rearrange("p (k b) -> p k b", b=rows)
            for b in range(rows):
                g = idxp.tile([P, K], i32, tag=f"gidx{b}")
                nc.vector.tensor_copy(g[:], ps3[:, :, b])
                gidx_b.append(g)
                if not scatter_write:
                    continue
                # scatter idx = (b*seq + k*P + p) + (gidx_f > BIG-1)*OUT_BIG
                sio = idxp.tile([P, K], i32, tag=f"sio{b}")
                nc.gpsimd.iota(
                    sio[:], [[P, K]], base=b * seq, channel_multiplier=1
                )
                mb = idxp.tile([P, K], i32, tag=f"mb{b}")
                nc.vector.tensor_scalar(
                    out=mb[:], in0=ps3[:, :, b], scalar1=BIG - 1.0,
                    scalar2=OUT_BIG, op0=Alu.is_gt, op1=Alu.mult,
                )
                s = idxp.tile([P, K], i32, tag=f"sidx{b}")
                nc.vector.tensor_tensor(s[:], sio[:], mb[:], op=Alu.add)
                sidx_b.append(s)

        data = ctx.enter_context(tc.tile_pool(name="data", bufs=bufs))

        # Per-token tiles: one [128, d] tile covers 128 consecutive output
        # rows (token j = b*seq + 128k + p on partition p). The indirect
        # offset AP is [128, 1]: the HW consumes exactly one index per
        # partition (one 4KB-row descriptor per partition).
        outv = out_d.ap().rearrange("(b k p) d -> b k p d", b=rows, p=P)
        for b in range(rows):
            for k in range(K):
                t = data.tile([P, d], f32)
                g_ap = gidx_b[b][:, k:k + 1]
                nc.gpsimd.indirect_dma_start(
                    out=t[:],
                    out_offset=None,
                    in_=tab_d.ap(),
                    in_offset=bass.IndirectOffsetOnAxis(ap=g_ap, axis=0),
                    bounds_check=ntab - 1 if skip_pads else None,
                    oob_is_err=not skip_pads,
                )
                if scatter_write:
                    s_ap = sidx_b[b][:, k:k + 1]
                    nc.gpsimd.indirect_dma_start(
                        out=out_d.ap(),
                        out_offset=bass.IndirectOffsetOnAxis(ap=s_ap, axis=0),
                        in_=t[:],
                        in_offset=None,
                        bounds_check=rows * seq - 1,
                        oob_is_err=False,
                    )
                else:
                    # alternate the two HWDGE rings (SP / ACT) for stores
                    seng = nc.sync if (b * K + k) % 2 == 0 else nc.scalar
                    seng.dma_start(outv[b, k], t[:])
    nc.compile()
    return nc


_nc_cache = {}

# Tuned configuration used by kernel()
KERNEL_CFG = dict(skip_pads=False, scatter_write=False)


def _get_nc(**cfg):
    key = tuple(sorted(cfg.items()))
    if key not in _nc_cache:
        _nc_cache[key] = build_nc(**cfg)
    return _nc_cache[key]


def run(input, weights, trace=False, **cfg):
    """Run the 8-core SPMD kernel; returns (output, BassKernelResults)."""
    tokens = np.ascontiguousarray(np.asarray(input).astype(np.int32))
    table = np.ascontiguousarray(np.asarray(weights, dtype=np.float32))
    assert tokens.shape == (BSZ, SEQ), tokens.shape
    assert table.shape == (NTAB, DIM), table.shape
    nc = _get_nc(**{**KERNEL_CFG, **cfg})
    in_maps = [
        {"tokens": np.ascontiguousarray(tokens[c * RPC:(c + 1) * RPC]),
         "table": table}
        for c in range(NCORES)
    ]
    res = bass_utils.run_bass_kernel_spmd(
        nc, in_maps, core_ids=list(range(NCORES)), trace=trace
    )
    out = np.concatenate(
        [r["out"].reshape(RPC, SEQ, DIM) for r in res.results], axis=0
    )
    return out, res


def kernel(input, weights):
    out, _ = run(input, weights)
    return out
